# revision 2
# baseline (speedup 1.0000x reference)
"""BiAttention kernel v3 for Trainium2, 8-core data-parallel SPMD.

Per batch:
  proj (f32r): x1p in SBUF [e,n]; x2p spilled to DRAM [e,m], streamed back.
  sim (f32r): simT[m,n]; G = bf16(exp(simT + NEG*m2 - 75)); s_row = ones@G.
  t = 64/s_row (K=1 bcast); A = fp8(G*t), B = fp8(G*t - A)    [m,n]
  transposes G->F0[n,m] (PE) staged, s_col += keep1@F0stg, F0 -> DRAM.
  u = 64/s_col; C = fp8(F0*keep1*u), D = fp8(. - C)           [n,m]
  attn_a = (A@(x2h+x2l) + B@x2h) * rec_a  (fp8 DoubleRow, m-pair contraction)
  attn_b = (C@(x1h+x1l) + D@x1h + m2i@blr) * rec_b
  rec_a = 1/(bf16(t)*s_row);  rec_b = 1/(bf16(u)*s_col + 2048*m2)
Value streams ride the Pool-engine SWDGE path to keep HWDGE free; outputs bf16.
"""
import sys

sys.path.insert(0, "/opt/trn_rl_repo")

import numpy as np
import ml_dtypes

import concourse.bass as bass  # noqa: F401
import concourse.bacc as bacc
import concourse.tile as tile
from concourse import mybir
from concourse.bass_utils import run_bass_kernel_spmd

B, Nn, Mm, D = 16, 2048, 2048, 1024
NCORES = 8
BPC = B // NCORES
P = 128
ET, DT, NT, MT = D // P, D // P, Nn // P, Mm // P
NEG = -2e20
C_SHIFT = 75.0
KSC = 64.0

F32 = mybir.dt.float32
F32R = mybir.dt.float32r
BF16 = mybir.dt.bfloat16
F8 = mybir.dt.float8e4
BF16_NP = ml_dtypes.bfloat16
F8_NP = ml_dtypes.float8_e4m3

Relu = mybir.ActivationFunctionType.Relu
Exp = mybir.ActivationFunctionType.Exp
Copy = mybir.ActivationFunctionType.Copy
Mult = mybir.AluOpType.mult
Add = mybir.AluOpType.add
Sub = mybir.AluOpType.subtract
DR = mybir.MatmulPerfMode.DoubleRow


def _emit(nc):
    dram = nc.dram_tensor
    x1t = dram("x1t", [BPC, DT, P, Nn], F32, kind="ExternalInput")
    x2t = dram("x2t", [BPC, DT, P, Mm], F32, kind="ExternalInput")
    w1t = dram("w1t", [DT, P, D], F32, kind="ExternalInput")
    w2t = dram("w2t", [DT, P, D], F32, kind="ExternalInput")
    b1c = dram("b1c", [P, ET], F32, kind="ExternalInput")
    b2c = dram("b2c", [P, ET], F32, kind="ExternalInput")
    x2h = dram("x2h", [BPC, MT, P, D], F8, kind="ExternalInput")
    x2l = dram("x2l", [BPC, MT, P, D], F8, kind="ExternalInput")
    x1h = dram("x1h", [BPC, NT, P, D], F8, kind="ExternalInput")
    x1l = dram("x1l", [BPC, NT, P, D], F8, kind="ExternalInput")
    x2mbc = dram("x2mbc", [BPC, P, MT], F32, kind="ExternalInput")
    keep1b = dram("keep1b", [BPC, P, NT], BF16, kind="ExternalInput")
    keep1f = dram("keep1f", [BPC, P, NT], F32, kind="ExternalInput")
    m2add = dram("m2add", [BPC, P, MT], F32, kind="ExternalInput")
    m2i = dram("m2i", [BPC, 1, Mm], BF16, kind="ExternalInput")
    blr = dram("blr", [BPC, 1, D], BF16, kind="ExternalInput")
    ident = dram("ident", [P, P], BF16, kind="ExternalInput")
    onescol = dram("onescol", [P, 1], BF16, kind="ExternalInput")
    k64row = dram("k64row", [1, P], F32, kind="ExternalInput")
    one11 = dram("one11", [1, 1], F32, kind="ExternalInput")
    one11b = dram("one11b", [1, 1], BF16, kind="ExternalInput")
    outa = dram("outa", [BPC, NT, P, D], BF16, kind="ExternalOutput")
    outb = dram("outb", [BPC, MT, P, D], BF16, kind="ExternalOutput")

    with tile.TileContext(nc) as tc:
        import contextlib

        with contextlib.ExitStack() as ctx:
            big = ctx.enter_context(tc.tile_pool(name="big", bufs=1))
            prs = ctx.enter_context(tc.tile_pool(name="projrhs", bufs=2))
            x2s = ctx.enter_context(tc.tile_pool(name="x2stream", bufs=2))
            f0s = ctx.enter_context(tc.tile_pool(name="f0stream", bufs=2))
            fstg = ctx.enter_context(tc.tile_pool(name="f0stg", bufs=2))
            pstg = ctx.enter_context(tc.tile_pool(name="pstg", bufs=2))
            vst = ctx.enter_context(tc.tile_pool(name="vals", bufs=2))
            ost = ctx.enter_context(tc.tile_pool(name="ostg", bufs=2))
            bcp = ctx.enter_context(tc.tile_pool(name="bcast", bufs=1))
            rows = ctx.enter_context(tc.tile_pool(name="rows", bufs=3))
            mrow = ctx.enter_context(tc.tile_pool(name="mrow", bufs=1))
            sml = ctx.enter_context(tc.tile_pool(name="small", bufs=1))
            cst = ctx.enter_context(tc.tile_pool(name="consts", bufs=1))
            dsc = ctx.enter_context(tc.tile_pool(name="dramscr", bufs=1, space="DRAM"))
            psA = ctx.enter_context(tc.tile_pool(name="psumA", bufs=8, space="PSUM"))
            tmpp = ctx.enter_context(tc.tile_pool(name="tmpp", bufs=3))

            b1c_t = cst.tile([P, ET], F32, tag="b1c")
            b2c_t = cst.tile([P, ET], F32, tag="b2c")
            ident_t = cst.tile([P, P], BF16, tag="ident")
            onescol_t = cst.tile([P, 1], BF16, tag="onescol")
            k64row_t = cst.tile([1, P], F32, tag="k64row")
            one11_t = cst.tile([1, 1], F32, tag="one11")
            one11b_t = cst.tile([1, 1], BF16, tag="one11b")
            nc.gpsimd.dma_start(out=b1c_t, in_=b1c.ap())
            nc.gpsimd.dma_start(out=b2c_t, in_=b2c.ap())
            nc.gpsimd.dma_start(out=ident_t, in_=ident.ap())
            nc.gpsimd.dma_start(out=onescol_t, in_=onescol.ap())
            nc.gpsimd.dma_start(out=k64row_t, in_=k64row.ap())
            nc.gpsimd.dma_start(out=one11_t, in_=one11.ap())
            nc.gpsimd.dma_start(out=one11b_t, in_=one11b.ap())

            for b in range(BPC):
                x2mbc_t = sml.tile([P, MT], F32, tag="x2mbc")
                keep1b_t = sml.tile([P, NT], BF16, tag="keep1b")
                keep1f_t = sml.tile([P, NT], F32, tag="keep1f")
                m2add_t = sml.tile([P, MT], F32, tag="m2add")
                nc.gpsimd.dma_start(
                    out=x2mbc_t, in_=x2mbc.ap()[b : b + 1].rearrange("o p t -> p (o t)")
                )
                nc.gpsimd.dma_start(
                    out=keep1b_t,
                    in_=keep1b.ap()[b : b + 1].rearrange("o p t -> p (o t)"),
                )
                nc.gpsimd.dma_start(
                    out=keep1f_t,
                    in_=keep1f.ap()[b : b + 1].rearrange("o p t -> p (o t)"),
                )
                nc.gpsimd.dma_start(
                    out=m2add_t, in_=m2add.ap()[b : b + 1].rearrange("o p t -> p (o t)")
                )
                m2i_t = mrow.tile([1, Mm], BF16, tag="m2i")
                blr_t = mrow.tile([1, D], BF16, tag="blr")
                nc.gpsimd.dma_start(
                    out=m2i_t, in_=m2i.ap()[b : b + 1].rearrange("o r m -> (o r) m")
                )
                nc.gpsimd.dma_start(
                    out=blr_t, in_=blr.ap()[b : b + 1].rearrange("o r m -> (o r) m")
                )

                x2pd = dsc.tile([ET, P, Mm], F32, tag="x2pd")
                f0d = dsc.tile([NT, P, Mm], BF16, tag="f0d")

                x1pA = big.tile([P, ET // 2, Nn], F32R, tag="s1", name="x1pA")
                x1pB = big.tile([P, ET // 2, Nn], F32R, tag="s2", name="x1pB")

                # ---- PHASE P: projections ----
                NCH = 256
                for proj in range(2):
                    wA = big.tile([P, DT // 2, D], F32R, tag="s3", name=f"w{proj}a")
                    wB = big.tile([P, DT // 2, D], F32R, tag="s4", name=f"w{proj}b")
                    wsrc = w1t if proj == 0 else w2t
                    for dt_ in range(DT):
                        nc.sync.dma_start(
                            out=(wA if dt_ < 4 else wB)[:, dt_ % 4, :],
                            in_=wsrc.ap()[dt_ : dt_ + 1]
                            .rearrange("t p e -> p (t e)")
                            .bitcast(F32R),
                        )
                    xt = x1t if proj == 0 else x2t
                    bc = b1c_t if proj == 0 else b2c_t
                    for nch in range(Nn // NCH):
                        rhs_t = prs.tile([P, DT, NCH], F32R, tag="prhs")
                        nc.sync.dma_start(
                            out=rhs_t,
                            in_=xt.ap()[
                                b : b + 1, :, :, nch * NCH : (nch + 1) * NCH
                            ]
                            .rearrange("o dt p n -> p (o dt) n")
                            .bitcast(F32R),
                        )
                        for et in range(ET):
                            ps = psA.tile([P, 512], F32, tag="ps")
                            for dt_ in range(DT):
                                nc.tensor.matmul(
                                    ps[:, :NCH],
                                    (wA if dt_ < 4 else wB)[
                                        :, dt_ % 4, et * P : (et + 1) * P
                                    ],
                                    rhs_t[:, dt_, :],
                                    start=(dt_ == 0),
                                    stop=(dt_ == DT - 1),
                                )
                            if proj == 0:
                                po = x1pA if et < 4 else x1pB
                                nc.scalar.activation(
                                    po[:, et % 4, nch * NCH : (nch + 1) * NCH],
                                    ps[:, :NCH],
                                    Relu,
                                    bias=bc[:, et : et + 1],
                                    scale=1.0,
                                )
                            else:
                                if et % 2 == 0:
                                    st2 = pstg.tile([P, 2, NCH], F32, tag="pstg")
                                nc.scalar.activation(
                                    st2[:, et % 2, :], ps[:, :NCH], Relu,
                                    bias=bc[:, et : et + 1], scale=1.0,
                                )
                                if et % 2 == 1:
                                    nc.gpsimd.dma_start(
                                        out=x2pd[
                                            et - 1 : et + 1,
                                            :,
                                            nch * NCH : (nch + 1) * NCH,
                                        ].rearrange("e p m -> p e m"),
                                        in_=st2,
                                    )

                # ---- PHASE S: simT[m,n] -> G bf16; s_row ----
                Ga = big.tile([P, MT // 2, Nn], BF16, tag="s3", name="Ga")
                Gb = big.tile([P, MT // 2, Nn], BF16, tag="s4", name="Gb")
                srow_ps = [
                    psA.tile([1, 512], F32, tag="ps", name=f"srow{j}") for j in range(4)
                ]
                for mt in range(MT):
                    x2ps = x2s.tile([P, ET, P], F32R, tag="x2ps")
                    nc.sync.dma_start(
                        out=x2ps,
                        in_=x2pd[:, :, mt * P : (mt + 1) * P]
                        .rearrange("e p m -> p e m")
                        .bitcast(F32R),
                    )
                    g_dst = Ga if mt < 8 else Gb
                    for c4 in range(4):
                        ps = psA.tile([P, 512], F32, tag="ps")
                        for et in range(ET):
                            x1p_src = x1pA if et < 4 else x1pB
                            nc.tensor.matmul(
                                ps,
                                x2ps[:, et, :],
                                x1p_src[:, et % 4, c4 * 512 : (c4 + 1) * 512],
                                start=(et == 0),
                                stop=(et == ET - 1),
                            )
                        nc.scalar.activation(
                            g_dst[:, mt % 8, c4 * 512 : (c4 + 1) * 512],
                            ps,
                            Exp,
                            bias=x2mbc_t[:, mt : mt + 1],
                            scale=1.0,
                        )
                        nc.tensor.matmul(
                            srow_ps[c4],
                            onescol_t,
                            g_dst[:, mt % 8, c4 * 512 : (c4 + 1) * 512],
                            start=(mt == 0),
                            stop=(mt == MT - 1),
                        )

                # ---- s_row -> t_bcast + rec_a ----
                t_bcast = bcp.tile([P, Nn], BF16, tag="tb", name="t_bcast")
                ps_fl = psA.tile([P, NT], F32, tag="ps", name="flipA")
                ps_flt = psA.tile([P, NT], F32, tag="ps", name="flipAt")
                for c4 in range(4):
                    srow_sb = rows.tile([1, 512], F32, tag="rch", name="srow_sb")
                    nc.vector.tensor_scalar(
                        out=srow_sb, in0=srow_ps[c4],
                        scalar1=1e-30, scalar2=None, op0=Add,
                    )
                    srow_rec = rows.tile([1, 512], F32, tag="rch", name="srow_rec")
                    nc.vector.reciprocal(srow_rec, srow_sb)
                    ps = psA.tile([P, 512], F32, tag="ps")
                    nc.tensor.matmul(
                        ps, k64row_t, srow_rec, start=True, stop=True
                    )
                    nc.scalar.activation(t_bcast[:, c4 * 512 : (c4 + 1) * 512], ps, Copy)
                    for jj in range(4):
                        j = c4 * 4 + jj
                        nc.tensor.matmul(
                            ps_fl[:, j : j + 1],
                            srow_sb[0:1, jj * P : (jj + 1) * P],
                            one11_t,
                            start=True,
                            stop=True,
                            skip_group_check=True,
                        )
                        nc.tensor.matmul(
                            ps_flt[:, j : j + 1],
                            t_bcast[0:1, j * P : (j + 1) * P],
                            one11b_t,
                            start=True,
                            stop=True,
                            skip_group_check=True,
                        )
                tflip = sml.tile([P, NT], F32, tag="tflip")
                nc.vector.tensor_copy(tflip, ps_flt)
                spflip = sml.tile([P, NT], F32, tag="spflip")
                nc.vector.tensor_tensor(out=spflip, in0=ps_fl, in1=tflip, op=Mult)
                rec_a = sml.tile([P, NT], F32, tag="rec_a")
                nc.vector.reciprocal(rec_a, spflip)

                # ---- PHASE W: A/B (sw-pipelined) + transposes -> F0 + s_col ----
                A_t = big.tile([P, MT, Nn], F8, tag="s1", name="A")
                B_t = big.tile([P, MT, Nn], F8, tag="s2", name="B")
                scol_ps = [
                    psA.tile([1, 512], F32, tag="ps", name=f"scol{j}") for j in range(4)
                ]
                tmps = {}
                for k in range(MT + 1):
                    if k < MT:
                        nt = k
                        for half in range(2):
                            ps_t8 = psA.tile([P, 8, P], BF16, tag="ps", name="ps_t8")
                            for j in range(8):
                                mt = half * 8 + j
                                g_src2 = Ga if mt < 8 else Gb
                                nc.tensor.transpose(
                                    ps_t8[:, j, :],
                                    g_src2[:, mt % 8, nt * P : (nt + 1) * P],
                                    ident_t,
                                )
                            fst = fstg.tile([P, 1024], BF16, tag="fstg")
                            if half == 0:
                                nc.scalar.activation(fst, ps_t8, Copy)
                            else:
                                nc.vector.tensor_copy(fst, ps_t8)
                            nc.sync.dma_start(
                                out=f0d[
                                    nt : nt + 1, :, half * 1024 : (half + 1) * 1024
                                ].rearrange("t p m -> p (t m)"),
                                in_=fst,
                            )
                            for q in range(2):
                                nc.tensor.matmul(
                                    scol_ps[half * 2 + q],
                                    keep1b_t[:, nt : nt + 1],
                                    fst[:, q * 512 : (q + 1) * 512],
                                    start=(nt == 0),
                                    stop=(nt == NT - 1),
                                )
                        g_src = Ga if k < 8 else Gb
                        tl = []
                        for hh in range(2):
                            sl = slice(hh * 1024, (hh + 1) * 1024)
                            tmp = tmpp.tile([P, 1024], BF16, tag="tmp", name="tmpAB")
                            nc.vector.tensor_tensor(
                                out=tmp, in0=g_src[:, k % 8, sl],
                                in1=t_bcast[:, sl], op=Mult,
                            )
                            nc.scalar.activation(A_t[:, k, sl], tmp, Copy)
                            tl.append(tmp)
                        tmps[k] = tl
                    if k > 0:
                        kk = k - 1
                        for hh in range(2):
                            sl = slice(hh * 1024, (hh + 1) * 1024)
                            nc.vector.scalar_tensor_tensor(
                                out=B_t[:, kk, sl], in0=tmps[kk][hh], scalar=1.0,
                                in1=A_t[:, kk, sl], op0=Mult, op1=Sub,
                            )
                        del tmps[kk]

                # ---- s_col -> u_bcast + rec_b ----
                u_bcast = bcp.tile([P, Mm], BF16, tag="tb", name="u_bcast")
                ps_fl2 = psA.tile([P, MT], F32, tag="ps", name="flipB")
                ps_fl2t = psA.tile([P, MT], F32, tag="ps", name="flipBt")
                for c4 in range(4):
                    scol_sb = rows.tile([1, 512], F32, tag="rch", name="scol_sb")
                    nc.vector.tensor_scalar(
                        out=scol_sb, in0=scol_ps[c4],
                        scalar1=1e-30, scalar2=None, op0=Add,
                    )
                    scol_rec = rows.tile([1, 512], F32, tag="rch", name="scol_rec")
                    nc.vector.reciprocal(scol_rec, scol_sb)
                    ps = psA.tile([P, 512], F32, tag="ps")
                    nc.tensor.matmul(
                        ps, k64row_t, scol_rec, start=True, stop=True
                    )
                    nc.scalar.activation(u_bcast[:, c4 * 512 : (c4 + 1) * 512], ps, Copy)
                    for jj in range(4):
                        j = c4 * 4 + jj
                        nc.tensor.matmul(
                            ps_fl2[:, j : j + 1],
                            scol_sb[0:1, jj * P : (jj + 1) * P],
                            one11_t,
                            start=True,
                            stop=True,
                            skip_group_check=True,
                        )
                        nc.tensor.matmul(
                            ps_fl2t[:, j : j + 1],
                            u_bcast[0:1, j * P : (j + 1) * P],
                            one11b_t,
                            start=True,
                            stop=True,
                            skip_group_check=True,
                        )
                uflip = sml.tile([P, MT], F32, tag="uflip")
                nc.vector.tensor_copy(uflip, ps_fl2t)
                spflip2 = sml.tile([P, MT], F32, tag="spflip2")
                nc.vector.tensor_tensor(out=spflip2, in0=ps_fl2, in1=uflip, op=Mult)
                spflip2b = sml.tile([P, MT], F32, tag="spflip2b")
                nc.vector.tensor_tensor(out=spflip2b, in0=spflip2, in1=m2add_t, op=Add)
                rec_b = sml.tile([P, MT], F32, tag="rec_b")
                nc.vector.reciprocal(rec_b, spflip2b)

                # ---- PHASE CD: C/D from re-streamed F0 (sw-pipelined) ----
                C_t = big.tile([P, NT, Mm], F8, tag="s3", name="C")
                D_t = big.tile([P, NT, Mm], F8, tag="s4", name="D")
                tmps2 = {}
                for k in range(NT + 1):
                    if k < NT:
                        tl = []
                        for hh in range(2):
                            sl = slice(hh * 1024, (hh + 1) * 1024)
                            f0in = f0s.tile([P, 1024], BF16, tag="f0in")
                            nc.scalar.dma_start(
                                out=f0in,
                                in_=f0d[k : k + 1, :, sl].rearrange("t p m -> p (t m)"),
                            )
                            tmp = tmpp.tile([P, 1024], BF16, tag="tmp", name="tmpCD")
                            nc.vector.scalar_tensor_tensor(
                                out=tmp, in0=f0in,
                                scalar=keep1f_t[:, k : k + 1],
                                in1=u_bcast[:, sl], op0=Mult, op1=Mult,
                            )
                            nc.scalar.activation(C_t[:, k, sl], tmp, Copy)
                            tl.append(tmp)
                        tmps2[k] = tl
                    if k > 0:
                        kk = k - 1
                        for hh in range(2):
                            sl = slice(hh * 1024, (hh + 1) * 1024)
                            nc.vector.scalar_tensor_tensor(
                                out=D_t[:, kk, sl], in0=tmps2[kk][hh], scalar=1.0,
                                in1=C_t[:, kk, sl], op0=Mult, op1=Sub,
                            )
                        del tmps2[kk]

                # ---- PHASE A: attn_a ----
                NG = 4
                for g0 in range(0, NT, NG):
                    nts = list(range(g0, g0 + NG))
                    psa = {}
                    for nt in nts:
                        for dchp in range(2):
                            psa[(nt, dchp)] = psA.tile(
                                [P, 512], F32, tag="ps", name=f"pa{nt}_{dchp}"
                            )
                    for mp in range(MT // 2):
                        if mp % 2 == 0:
                            vh = vst.tile([P, 2, D], F8, tag="vh")
                            vl = vst.tile([P, 2, D], F8, tag="vl")
                        else:
                            vh = pstg.tile([P, 2, D], F8, tag="vhx")
                            vl = x2s.tile([P, 2, D], F8, tag="vlx")
                        for v_t, src, eng in ((vh, x2h, nc.sync), (vl, x2l, nc.gpsimd)):
                            eng.dma_start(
                                out=v_t,
                                in_=src.ap()[
                                    b : b + 1, 2 * mp : 2 * mp + 2, :, :
                                ].rearrange("o t p d -> p (o t) d"),
                            )
                        for term in range(3):
                            for nt in nts:
                                lA = A_t[:, 2 * mp : 2 * mp + 2, nt * P : (nt + 1) * P]
                                lB = B_t[:, 2 * mp : 2 * mp + 2, nt * P : (nt + 1) * P]
                                lw = lA if term < 2 else lB
                                vv = vh if term != 1 else vl
                                for dchp in range(2):
                                    pst = psa[(nt, dchp)]
                                    for sub in range(2):
                                        d0 = dchp * 512 + sub * 256
                                        po = pst[:, sub * 256 : (sub + 1) * 256]
                                        first = mp == 0 and sub == 0 and term == 0
                                        nc.tensor.matmul(
                                            po, lw, vv[:, :, d0 : d0 + 256],
                                            start=first,
                                            stop=(
                                                mp == MT // 2 - 1
                                                and sub == 1
                                                and term == 2
                                            ),
                                            perf_mode=DR,
                                            skip_group_check=not first,
                                        )
                    for nt in nts:
                        for dchp in range(2):
                            st = ost.tile([P, 512], BF16, tag="ostg")
                            nc.scalar.activation(
                                st, psa[(nt, dchp)], Copy,
                                scale=rec_a[:, nt : nt + 1],
                            )
                            nc.scalar.dma_start(
                                out=outa.ap()[
                                    b : b + 1,
                                    nt : nt + 1,
                                    :,
                                    dchp * 512 : (dchp + 1) * 512,
                                ].rearrange("o t p d -> p (o t d)"),
                                in_=st,
                            )

                # ---- PHASE B: attn_b ----
                for g0 in range(0, MT, NG):
                    mts = list(range(g0, g0 + NG))
                    psb = {}
                    for mt in mts:
                        for dchp in range(2):
                            psb[(mt, dchp)] = psA.tile(
                                [P, 512], F32, tag="ps", name=f"pb{mt}_{dchp}"
                            )
                    for np_ in range(NT // 2):
                        if np_ % 2 == 0:
                            vh = vst.tile([P, 2, D], F8, tag="vh")
                            vl = vst.tile([P, 2, D], F8, tag="vl")
                        else:
                            vh = pstg.tile([P, 2, D], F8, tag="vhx")
                            vl = x2s.tile([P, 2, D], F8, tag="vlx")
                        for v_t, src, eng in ((vh, x1h, nc.sync), (vl, x1l, nc.gpsimd)):
                            eng.dma_start(
                                out=v_t,
                                in_=src.ap()[
                                    b : b + 1, 2 * np_ : 2 * np_ + 2, :, :
                                ].rearrange("o t p d -> p (o t) d"),
                            )
                        for term in range(3):
                            for mt in mts:
                                lC = C_t[:, 2 * np_ : 2 * np_ + 2, mt * P : (mt + 1) * P]
                                lD = D_t[:, 2 * np_ : 2 * np_ + 2, mt * P : (mt + 1) * P]
                                lw = lC if term < 2 else lD
                                vv = vh if term != 1 else vl
                                for dchp in range(2):
                                    pst = psb[(mt, dchp)]
                                    for sub in range(2):
                                        d0 = dchp * 512 + sub * 256
                                        po = pst[:, sub * 256 : (sub + 1) * 256]
                                        first = np_ == 0 and sub == 0 and term == 0
                                        nc.tensor.matmul(
                                            po, lw, vv[:, :, d0 : d0 + 256],
                                            start=first, stop=False,
                                            perf_mode=DR,
                                            skip_group_check=not first,
                                        )
                    for mt in mts:
                        for dchp in range(2):
                            pst = psb[(mt, dchp)]
                            for sub in range(2):
                                nc.tensor.matmul(
                                    pst[:, sub * 256 : (sub + 1) * 256],
                                    m2i_t[0:1, mt * P : (mt + 1) * P],
                                    blr_t[
                                        0:1,
                                        dchp * 512 + sub * 256 : dchp * 512
                                        + (sub + 1) * 256,
                                    ],
                                    start=False,
                                    stop=(sub == 1),
                                    skip_group_check=True,
                                )
                            st = ost.tile([P, 512], BF16, tag="ostg")
                            nc.scalar.activation(
                                st, pst, Copy, scale=rec_b[:, mt : mt + 1]
                            )
                            nc.scalar.dma_start(
                                out=outb.ap()[
                                    b : b + 1,
                                    mt : mt + 1,
                                    :,
                                    dchp * 512 : (dchp + 1) * 512,
                                ].rearrange("o t p d -> p (o t d)"),
                                in_=st,
                            )


_NC_CACHE = None


def _get_nc():
    global _NC_CACHE
    if _NC_CACHE is None:
        nc = bacc.Bacc("TRN2", target_bir_lowering=False, debug=False)
        _emit(nc)
        nc.compile()
        _NC_CACHE = nc
    return _NC_CACHE


def _prep_in_maps(x1, x1_mask, x2, x2_mask, W1, b1, W2, b2):
    f32 = np.float32
    x1 = np.ascontiguousarray(x1, f32)
    x2 = np.ascontiguousarray(x2, f32)
    W1 = np.ascontiguousarray(W1, f32)
    W2 = np.ascontiguousarray(W2, f32)
    b1 = np.asarray(b1, f32)
    b2 = np.asarray(b2, f32)
    m1 = np.asarray(x1_mask, bool)
    m2 = np.asarray(x2_mask, bool)

    w1t_ = np.ascontiguousarray(W1.T).reshape(DT, P, D)
    w2t_ = np.ascontiguousarray(W2.T).reshape(DT, P, D)
    b1c_ = np.ascontiguousarray(b1.reshape(ET, P).T)
    b2c_ = np.ascontiguousarray(b2.reshape(ET, P).T)
    ident_ = np.eye(P, dtype=BF16_NP)
    onescol_ = np.ones((P, 1), BF16_NP)
    k64row_ = np.full((1, P), KSC, f32)
    one11_ = np.ones((1, 1), f32)
    one11b_ = np.ones((1, 1), BF16_NP)

    in_maps = []
    for c in range(NCORES):
        sl = slice(c * BPC, (c + 1) * BPC)
        x1c, x2c = x1[sl], x2[sl]
        m1c, m2c = m1[sl], m2[sl]
        x1tc = np.ascontiguousarray(x1c.transpose(0, 2, 1)).reshape(BPC, DT, P, Nn)
        x2tc = np.ascontiguousarray(x2c.transpose(0, 2, 1)).reshape(BPC, DT, P, Mm)
        x1z = np.where(m1c[:, :, None], 0.0, x1c)
        x1h_ = x1z.astype(F8_NP)
        x1l_ = (x1z - x1h_.astype(f32)).astype(F8_NP)
        x2h_ = x2c.astype(F8_NP)
        x2l_ = (x2c - x2h_.astype(f32)).astype(F8_NP)
        x2mb = np.where(m2c, np.float64(NEG), 0.0) - C_SHIFT
        x2mbc_ = np.ascontiguousarray(
            x2mb.astype(f32).reshape(BPC, MT, P).transpose(0, 2, 1)
        )
        keep1 = (~m1c).astype(f32)
        keep1bc = np.ascontiguousarray(
            keep1.reshape(BPC, NT, P).transpose(0, 2, 1)
        ).astype(BF16_NP)
        keep1fc = np.ascontiguousarray(keep1.reshape(BPC, NT, P).transpose(0, 2, 1))
        m2addc = np.ascontiguousarray(
            (2048.0 * m2c.astype(f32)).reshape(BPC, MT, P).transpose(0, 2, 1)
        )
        m2i_ = m2c.astype(BF16_NP).reshape(BPC, 1, Mm)
        blrow = x1c.sum(axis=1, dtype=np.float64).astype(BF16_NP).reshape(BPC, 1, D)
        in_maps.append(
            {
                "x1t": x1tc,
                "x2t": x2tc,
                "w1t": w1t_,
                "w2t": w2t_,
                "b1c": b1c_,
                "b2c": b2c_,
                "x2h": np.ascontiguousarray(x2h_).reshape(BPC, MT, P, D),
                "x2l": np.ascontiguousarray(x2l_).reshape(BPC, MT, P, D),
                "x1h": np.ascontiguousarray(x1h_).reshape(BPC, NT, P, D),
                "x1l": np.ascontiguousarray(x1l_).reshape(BPC, NT, P, D),
                "x2mbc": x2mbc_,
                "keep1b": keep1bc,
                "keep1f": keep1fc,
                "m2add": m2addc,
                "m2i": m2i_,
                "blr": blrow,
                "ident": ident_,
                "onescol": onescol_,
                "k64row": k64row_,
                "one11": one11_,
                "one11b": one11b_,
            }
        )
    return in_maps


def kernel(x1, x1_mask, x2, x2_mask, W1, b1, W2, b2, _trace=False):
    nc = _get_nc()
    in_maps = _prep_in_maps(x1, x1_mask, x2, x2_mask, W1, b1, W2, b2)
    res = run_bass_kernel_spmd(nc, in_maps, core_ids=list(range(NCORES)), trace=_trace)
    attn_a = np.empty((B, Nn, D), np.float32)
    attn_b = np.empty((B, Mm, D), np.float32)
    for c in range(NCORES):
        sl = slice(c * BPC, (c + 1) * BPC)
        attn_a[sl] = res.results[c]["outa"].astype(np.float32).reshape(BPC, Nn, D)
        attn_b[sl] = res.results[c]["outb"].astype(np.float32).reshape(BPC, Mm, D)
    if _trace:
        kernel._last_exec_time_ns = res.exec_time_ns
        kernel._last_results = res
    return attn_a, attn_b
